# revision 27
# baseline (speedup 1.0000x reference)
"""Mixtral MoE MLP (T=8192, H=2048, I=4096, E=8, top-2) on 8 TRN2 NeuronCores.

Strategy: expert-parallel. The router (tiny: T*H*E macs) runs on host in
float64; tokens are dispatched to the core owning each selected expert
(all-to-all realized as the host-side shard step), each core runs the
SwiGLU expert MLP as two fp16 GEMMs with fp32 PSUM accumulation, and the
host combines the per-expert outputs with the renormalized top-2 weights.

Device kernel per core e (C = padded token capacity):
  gate_upT[2I, C] = ws[e] @ x_eT      (weights stationary, tokens moving)
  actT[I, C]      = silu(gateT) * upT (ACT sigmoid + DVE muls, f32 -> fp16)
  yT[H, C]        = w2s[e] @ actT
All tensors live transposed (feature-major) so both GEMMs keep weights as
the stationary operand; tokens stream through in near-uniform <=512-wide
moving blocks (wide enough to hide LDWEIGHTS), two blocks per stripe, so
x/act stripes stay in SBUF while weights stream from HBM once per stripe.
"""

import numpy as np

T, H, I, E = 8192, 2048, 4096, 8
TOP_K = 2
P = 128
KH = H // P            # 16  K-tiles for GEMM1 (contraction over H)
KI = I // P            # 32  K-tiles for GEMM2 (contraction over I)
NPAIR = I // P         # 32  gate/up 128-row pair blocks
NH = H // P            # 16  output row blocks of GEMM2
BLOCK = 512            # moving-operand / PSUM bank width

_module_cache = {}


def _stripes(C):
    """Split [0, C) into near-uniform blocks of <= BLOCK tokens (so every
    matmul is long enough to hide its LDWEIGHTS), grouped 2 per stripe."""
    n_blocks = max(1, -(-C // BLOCK))
    base, rem = divmod(C, n_blocks)
    widths = [base + 1] * rem + [base] * (n_blocks - rem)
    out = []
    off = 0
    for i in range(0, len(widths), 2):
        ws = widths[i:i + 2]
        blocks = []
        boff = 0
        for w in ws:
            blocks.append((boff, w))
            boff += w
        out.append((off, boff, blocks))
        off += boff
    return out


def _build_module(C, use_silu=True):
    import concourse.mybir as mybir
    import concourse.tile as tile
    from concourse import bacc
    from contextlib import ExitStack

    fp16 = mybir.dt.float16
    fp32 = mybir.dt.float32

    nc = bacc.Bacc("TRN2", target_bir_lowering=False, debug=False)

    xt = nc.dram_tensor("xt", [P, KH, C], fp16, kind="ExternalInput")
    w1 = nc.dram_tensor("w1", [NPAIR, P, KH, 2 * P], fp16, kind="ExternalInput")
    w2 = nc.dram_tensor("w2", [NH, P, KI, P], fp16, kind="ExternalInput")
    yt = nc.dram_tensor("yt", [P, NH, C], fp32, kind="ExternalOutput")

    # CoreSim only implements Sigmoid; hardware has native Silu (one fewer
    # DVE op on the PSUM drain path). sim_test flips this to False.
    act_fn = (mybir.ActivationFunctionType.Silu if use_silu
              else mybir.ActivationFunctionType.Sigmoid)

    with tile.TileContext(nc) as tc, ExitStack() as ctx:
        xpool = ctx.enter_context(tc.tile_pool(name="xs", bufs=2))
        apool = ctx.enter_context(tc.tile_pool(name="act", bufs=1))
        w1pool = ctx.enter_context(tc.tile_pool(name="w1p", bufs=2))
        w2pool = ctx.enter_context(tc.tile_pool(name="w2p", bufs=2))
        tpool = ctx.enter_context(tc.tile_pool(name="tmp", bufs=3))
        ypool = ctx.enter_context(tc.tile_pool(name="yst", bufs=3))
        ps1 = ctx.enter_context(tc.tile_pool(name="ps1", bufs=2, space="PSUM"))
        ps2 = ctx.enter_context(tc.tile_pool(name="ps2", bufs=2, space="PSUM"))

        for s_off, s_w, blocks in _stripes(C):
            xs = xpool.tile([P, KH, s_w], fp16)
            for bi, (b_off, b_w) in enumerate(blocks):
                # split by K so the first chains' operands land sooner; the
                # very first block is fully k-granular to start the PE asap
                step = 1 if (s_off == 0 and bi == 0) else KH // 2
                for k0 in range(0, KH, step):
                    nc.sync.dma_start(
                        xs[:, k0:k0 + step, b_off:b_off + b_w],
                        xt[:, k0:k0 + step,
                           s_off + b_off:s_off + b_off + b_w])
            actT = apool.tile([P, KI, s_w], fp16)

            # GEMM1 + SwiGLU: actT[:, p, :] = silu(g_p) * u_p
            for p in range(NPAIR):
                w1t = w1pool.tile([P, KH, 2 * P], fp16)
                # separate engine queue from the xs loads so the first
                # stripe's x and w transfers run in parallel; the first
                # pair is k-granular to start the PE asap
                wstep = 1 if (s_off == 0 and p == 0) else KH // 2
                for k0 in range(0, KH, wstep):
                    nc.scalar.dma_start(w1t[:, k0:k0 + wstep, :],
                                        w1[p, :, k0:k0 + wstep, :])
                for b_off, b_w in blocks:
                    pg = ps1.tile([P, b_w], fp32)
                    pu = ps1.tile([P, b_w], fp32)
                    for k in range(KH):
                        nc.tensor.matmul(
                            pg[:], w1t[:, k, 0:P], xs[:, k, b_off:b_off + b_w],
                            start=(k == 0), stop=(k == KH - 1))
                    for k in range(KH):
                        nc.tensor.matmul(
                            pu[:], w1t[:, k, P:2 * P], xs[:, k, b_off:b_off + b_w],
                            start=(k == 0), stop=(k == KH - 1))
                    tmp = tpool.tile([P, b_w], fp32)
                    nc.scalar.activation(tmp[:], pg[:], act_fn)
                    if use_silu:
                        nc.vector.tensor_mul(
                            actT[:, p, b_off:b_off + b_w], tmp[:], pu[:])
                    else:
                        tmp2 = tpool.tile([P, b_w], fp32)
                        nc.vector.tensor_mul(tmp2[:], tmp[:], pg[:])
                        nc.vector.tensor_mul(
                            actT[:, p, b_off:b_off + b_w], tmp2[:], pu[:])

            # GEMM2: yT[h-block, :] = sum_k2 w2[h].T @ actT
            for h in range(NH):
                w2t = w2pool.tile([P, KI, P], fp16)
                nc.sync.dma_start(w2t[:], w2[h])
                for b_off, b_w in blocks:
                    ps = ps2.tile([P, b_w], fp32)
                    for k2 in range(KI):
                        nc.tensor.matmul(
                            ps[:], w2t[:, k2, :], actT[:, k2, b_off:b_off + b_w],
                            start=(k2 == 0), stop=(k2 == KI - 1))
                    ys = ypool.tile([P, b_w], fp32)
                    nc.vector.tensor_copy(ys[:], ps[:])
                    nc.sync.dma_start(
                        yt[:, h, s_off + b_off:s_off + b_off + b_w], ys[:])

    nc.compile()
    return nc


def _route(hidden_states, router_w):
    """Replicate reference routing: softmax -> top-2 -> renormalize."""
    logits = hidden_states.astype(np.float64) @ router_w.astype(np.float64).T
    order = np.argsort(-logits, axis=1, kind="stable")
    top2 = order[:, :TOP_K]                                   # [T, 2]
    m = logits.max(axis=1, keepdims=True)
    p = np.exp(logits - m)
    p /= p.sum(axis=1, keepdims=True)
    w = np.take_along_axis(p, top2, axis=1)
    w = w / w.sum(axis=1, keepdims=True)                      # [T, 2]
    return top2, w


def _prep_w1(ws_e):
    # ws_e: [2I, H] fp32 -> [NPAIR, P(part=H%128), KH, 256] fp16
    w16 = ws_e.astype(np.float16)
    out = np.empty((NPAIR, P, KH, 2 * P), dtype=np.float16)
    out[:, :, :, :P] = w16[:I].reshape(NPAIR, P, KH, P).transpose(0, 3, 2, 1)
    out[:, :, :, P:] = w16[I:].reshape(NPAIR, P, KH, P).transpose(0, 3, 2, 1)
    return out


def _prep_w2(w2s_e):
    # w2s_e: [H, I] fp32 -> [NH, P(part=I%128), KI, P(col=H%128)] fp16
    w16 = w2s_e.astype(np.float16)
    return np.ascontiguousarray(
        w16.reshape(NH, P, KI, P).transpose(0, 3, 2, 1))


def _prep_x(x_pad16):
    # x_pad16: [C, H] fp16 -> [P(part), KH, C] fp16
    return np.ascontiguousarray(
        x_pad16.T.reshape(KH, P, -1).transpose(1, 0, 2))


def _ensure_ntff_hook():
    """Register the axon NTFF profile hook if the image's antenv lacks it."""
    import sys, types
    try:
        from antenv.axon_hooks import get_axon_ntff_profile_hook  # noqa: F401
        return
    except ImportError:
        pass
    try:
        from trn_agent_boot.trn_boot import _ntff_profile_via_ctypes
        hook = _ntff_profile_via_ctypes("/opt/axon/libaxon_pjrt.so")
    except Exception:
        hook = None
    mod = types.ModuleType("antenv.axon_hooks")
    mod.get_axon_ntff_profile_hook = lambda: hook
    mod.set_axon_ntff_profile_hook = lambda h: None
    sys.modules["antenv.axon_hooks"] = mod


def _run(hidden_states, router_w, ws, w2s, trace=False):
    from concourse.bass_utils import run_bass_kernel_spmd

    if trace:
        _ensure_ntff_hook()

    hidden_states = np.asarray(hidden_states, dtype=np.float32)
    router_w = np.asarray(router_w, dtype=np.float32)
    ws = np.asarray(ws, dtype=np.float32)
    w2s = np.asarray(w2s, dtype=np.float32)

    top2, topw = _route(hidden_states, router_w)

    tok_idx = []        # per-expert token ids
    tok_w = []          # per-expert combine weights
    for e in range(E):
        rows, which = np.nonzero(top2 == e)
        tok_idx.append(rows)
        tok_w.append(topw[rows, which])

    max_cnt = max(1, max(len(ix) for ix in tok_idx))
    C = -(-max_cnt // 8) * 8                    # round up to 8
    C = max(C, 256)

    if C not in _module_cache:
        _module_cache[C] = _build_module(C)
    nc = _module_cache[C]

    hidden16 = hidden_states.astype(np.float16)
    in_maps = []
    for e in range(E):
        rows = tok_idx[e]
        x_pad = np.zeros((C, H), dtype=np.float16)
        x_pad[:len(rows)] = hidden16[rows]
        in_maps.append({
            "xt": _prep_x(x_pad),
            "w1": _prep_w1(ws[e]),
            "w2": _prep_w2(w2s[e]),
        })

    res = run_bass_kernel_spmd(nc, in_maps, core_ids=list(range(E)),
                               trace=trace)

    out = np.zeros(hidden_states.shape, dtype=np.float32)
    for e in range(E):
        rows = tok_idx[e]
        if not len(rows):
            continue
        y = res.results[e]["yt"]                # [P, NH, C] fp32
        y = y.transpose(1, 0, 2).reshape(H, C).T  # [C, H]
        out[rows] += tok_w[e][:, None].astype(np.float32) * y[:len(rows)]
    return out, res


def kernel(hidden_states, router_w, ws, w2s):
    out, _ = _run(hidden_states, router_w, ws, w2s, trace=False)
    return out


# revision 29
# speedup vs baseline: 1.0369x; 1.0369x over previous
"""Mixtral MoE MLP (T=8192, H=2048, I=4096, E=8, top-2) on 8 TRN2 NeuronCores.

Strategy: expert-parallel. The router (tiny: T*H*E macs) runs on host in
float64; tokens are dispatched to the core owning each selected expert
(all-to-all realized as the host-side shard step), each core runs the
SwiGLU expert MLP as two fp16 GEMMs with fp32 PSUM accumulation, and the
host combines the per-expert outputs with the renormalized top-2 weights.

Device kernel per core e (C = padded token capacity):
  gate_upT[2I, C] = ws[e] @ x_eT      (weights stationary, tokens moving)
  actT[I, C]      = silu(gateT) * upT (ACT sigmoid + DVE muls, f32 -> fp16)
  yT[H, C]        = w2s[e] @ actT
All tensors live transposed (feature-major) so both GEMMs keep weights as
the stationary operand; tokens stream through in near-uniform <=512-wide
moving blocks (wide enough to hide LDWEIGHTS), two blocks per stripe, so
x/act stripes stay in SBUF while weights stream from HBM once per stripe.
"""

import numpy as np

T, H, I, E = 8192, 2048, 4096, 8
TOP_K = 2
P = 128
KH = H // P            # 16  K-tiles for GEMM1 (contraction over H)
KI = I // P            # 32  K-tiles for GEMM2 (contraction over I)
NPAIR = I // P         # 32  gate/up 128-row pair blocks
NH = H // P            # 16  output row blocks of GEMM2
BLOCK = 512            # moving-operand / PSUM bank width

_module_cache = {}


def _stripes(C):
    """Split [0, C) into near-uniform blocks of <= BLOCK tokens (so every
    matmul is long enough to hide its LDWEIGHTS), grouped 2 per stripe."""
    n_blocks = max(1, -(-C // BLOCK))
    base, rem = divmod(C, n_blocks)
    widths = [base + 1] * rem + [base] * (n_blocks - rem)
    out = []
    off = 0
    for i in range(0, len(widths), 2):
        ws = widths[i:i + 2]
        blocks = []
        boff = 0
        for w in ws:
            blocks.append((boff, w))
            boff += w
        out.append((off, boff, blocks))
        off += boff
    return out


def _build_module(C, use_silu=True):
    import concourse.mybir as mybir
    import concourse.tile as tile
    from concourse import bacc
    from contextlib import ExitStack

    fp16 = mybir.dt.float16
    fp32 = mybir.dt.float32

    nc = bacc.Bacc("TRN2", target_bir_lowering=False, debug=False)

    xt = nc.dram_tensor("xt", [P, KH, C], fp16, kind="ExternalInput")
    w1 = nc.dram_tensor("w1", [NPAIR, P, KH, 2 * P], fp16, kind="ExternalInput")
    w2 = nc.dram_tensor("w2", [NH, P, KI, P], fp16, kind="ExternalInput")
    yt = nc.dram_tensor("yt", [P, NH, C], fp32, kind="ExternalOutput")

    # CoreSim only implements Sigmoid; hardware has native Silu (one fewer
    # DVE op on the PSUM drain path). sim_test flips this to False.
    act_fn = (mybir.ActivationFunctionType.Silu if use_silu
              else mybir.ActivationFunctionType.Sigmoid)

    with tile.TileContext(nc) as tc, ExitStack() as ctx:
        xpool = ctx.enter_context(tc.tile_pool(name="xs", bufs=2))
        apool = ctx.enter_context(tc.tile_pool(name="act", bufs=1))
        w1pool = ctx.enter_context(tc.tile_pool(name="w1p", bufs=2))
        w2pool = ctx.enter_context(tc.tile_pool(name="w2p", bufs=2))
        tpool = ctx.enter_context(tc.tile_pool(name="tmp", bufs=3))
        ypool = ctx.enter_context(tc.tile_pool(name="yst", bufs=3))
        ps1 = ctx.enter_context(tc.tile_pool(name="ps1", bufs=2, space="PSUM"))
        ps2 = ctx.enter_context(tc.tile_pool(name="ps2", bufs=2, space="PSUM"))

        for s_off, s_w, blocks in _stripes(C):
            xs = xpool.tile([P, KH, s_w], fp16)
            for bi, (b_off, b_w) in enumerate(blocks):
                # split by K so the first chains' operands land sooner; the
                # first stripe is fully k-granular to start the PE asap
                step = 1 if s_off == 0 else KH // 2
                for k0 in range(0, KH, step):
                    nc.sync.dma_start(
                        xs[:, k0:k0 + step, b_off:b_off + b_w],
                        xt[:, k0:k0 + step,
                           s_off + b_off:s_off + b_off + b_w])
            actT = apool.tile([P, KI, s_w], fp16)

            # GEMM1 + SwiGLU: actT[:, p, :] = silu(g_p) * u_p
            for p in range(NPAIR):
                w1t = w1pool.tile([P, KH, 2 * P], fp16)
                # separate engine queue from the xs loads so the first
                # stripe's x and w transfers run in parallel; the first
                # pair is k-granular to start the PE asap
                wstep = 1 if (s_off == 0 and p <= 1) else KH // 2
                for k0 in range(0, KH, wstep):
                    nc.scalar.dma_start(w1t[:, k0:k0 + wstep, :],
                                        w1[p, :, k0:k0 + wstep, :])
                for b_off, b_w in blocks:
                    pg = ps1.tile([P, b_w], fp32)
                    pu = ps1.tile([P, b_w], fp32)
                    for k in range(KH):
                        nc.tensor.matmul(
                            pg[:], w1t[:, k, 0:P], xs[:, k, b_off:b_off + b_w],
                            start=(k == 0), stop=(k == KH - 1))
                    for k in range(KH):
                        nc.tensor.matmul(
                            pu[:], w1t[:, k, P:2 * P], xs[:, k, b_off:b_off + b_w],
                            start=(k == 0), stop=(k == KH - 1))
                    tmp = tpool.tile([P, b_w], fp32)
                    nc.scalar.activation(tmp[:], pg[:], act_fn)
                    if use_silu:
                        nc.vector.tensor_mul(
                            actT[:, p, b_off:b_off + b_w], tmp[:], pu[:])
                    else:
                        tmp2 = tpool.tile([P, b_w], fp32)
                        nc.vector.tensor_mul(tmp2[:], tmp[:], pg[:])
                        nc.vector.tensor_mul(
                            actT[:, p, b_off:b_off + b_w], tmp2[:], pu[:])

            # GEMM2: yT[h-block, :] = sum_k2 w2[h].T @ actT
            for h in range(NH):
                w2t = w2pool.tile([P, KI, P], fp16)
                nc.sync.dma_start(w2t[:], w2[h])
                for b_off, b_w in blocks:
                    ps = ps2.tile([P, b_w], fp32)
                    for k2 in range(KI):
                        nc.tensor.matmul(
                            ps[:], w2t[:, k2, :], actT[:, k2, b_off:b_off + b_w],
                            start=(k2 == 0), stop=(k2 == KI - 1))
                    ys = ypool.tile([P, b_w], fp32)
                    nc.vector.tensor_copy(ys[:], ps[:])
                    nc.sync.dma_start(
                        yt[:, h, s_off + b_off:s_off + b_off + b_w], ys[:])

    nc.compile()
    return nc


def _route(hidden_states, router_w):
    """Replicate reference routing: softmax -> top-2 -> renormalize."""
    logits = hidden_states.astype(np.float64) @ router_w.astype(np.float64).T
    order = np.argsort(-logits, axis=1, kind="stable")
    top2 = order[:, :TOP_K]                                   # [T, 2]
    m = logits.max(axis=1, keepdims=True)
    p = np.exp(logits - m)
    p /= p.sum(axis=1, keepdims=True)
    w = np.take_along_axis(p, top2, axis=1)
    w = w / w.sum(axis=1, keepdims=True)                      # [T, 2]
    return top2, w


def _prep_w1(ws_e):
    # ws_e: [2I, H] fp32 -> [NPAIR, P(part=H%128), KH, 256] fp16
    w16 = ws_e.astype(np.float16)
    out = np.empty((NPAIR, P, KH, 2 * P), dtype=np.float16)
    out[:, :, :, :P] = w16[:I].reshape(NPAIR, P, KH, P).transpose(0, 3, 2, 1)
    out[:, :, :, P:] = w16[I:].reshape(NPAIR, P, KH, P).transpose(0, 3, 2, 1)
    return out


def _prep_w2(w2s_e):
    # w2s_e: [H, I] fp32 -> [NH, P(part=I%128), KI, P(col=H%128)] fp16
    w16 = w2s_e.astype(np.float16)
    return np.ascontiguousarray(
        w16.reshape(NH, P, KI, P).transpose(0, 3, 2, 1))


def _prep_x(x_pad16):
    # x_pad16: [C, H] fp16 -> [P(part), KH, C] fp16
    return np.ascontiguousarray(
        x_pad16.T.reshape(KH, P, -1).transpose(1, 0, 2))


def _ensure_ntff_hook():
    """Register the axon NTFF profile hook if the image's antenv lacks it."""
    import sys, types
    try:
        from antenv.axon_hooks import get_axon_ntff_profile_hook  # noqa: F401
        return
    except ImportError:
        pass
    try:
        from trn_agent_boot.trn_boot import _ntff_profile_via_ctypes
        hook = _ntff_profile_via_ctypes("/opt/axon/libaxon_pjrt.so")
    except Exception:
        hook = None
    mod = types.ModuleType("antenv.axon_hooks")
    mod.get_axon_ntff_profile_hook = lambda: hook
    mod.set_axon_ntff_profile_hook = lambda h: None
    sys.modules["antenv.axon_hooks"] = mod


def _run(hidden_states, router_w, ws, w2s, trace=False):
    from concourse.bass_utils import run_bass_kernel_spmd

    if trace:
        _ensure_ntff_hook()

    hidden_states = np.asarray(hidden_states, dtype=np.float32)
    router_w = np.asarray(router_w, dtype=np.float32)
    ws = np.asarray(ws, dtype=np.float32)
    w2s = np.asarray(w2s, dtype=np.float32)

    top2, topw = _route(hidden_states, router_w)

    tok_idx = []        # per-expert token ids
    tok_w = []          # per-expert combine weights
    for e in range(E):
        rows, which = np.nonzero(top2 == e)
        tok_idx.append(rows)
        tok_w.append(topw[rows, which])

    max_cnt = max(1, max(len(ix) for ix in tok_idx))
    C = -(-max_cnt // 8) * 8                    # round up to 8
    C = max(C, 256)

    if C not in _module_cache:
        _module_cache[C] = _build_module(C)
    nc = _module_cache[C]

    hidden16 = hidden_states.astype(np.float16)
    in_maps = []
    for e in range(E):
        rows = tok_idx[e]
        x_pad = np.zeros((C, H), dtype=np.float16)
        x_pad[:len(rows)] = hidden16[rows]
        in_maps.append({
            "xt": _prep_x(x_pad),
            "w1": _prep_w1(ws[e]),
            "w2": _prep_w2(w2s[e]),
        })

    res = run_bass_kernel_spmd(nc, in_maps, core_ids=list(range(E)),
                               trace=trace)

    out = np.zeros(hidden_states.shape, dtype=np.float32)
    for e in range(E):
        rows = tok_idx[e]
        if not len(rows):
            continue
        y = res.results[e]["yt"]                # [P, NH, C] fp32
        y = y.transpose(1, 0, 2).reshape(H, C).T  # [C, H]
        out[rows] += tok_w[e][:, None].astype(np.float32) * y[:len(rows)]
    return out, res


def kernel(hidden_states, router_w, ws, w2s):
    out, _ = _run(hidden_states, router_w, ws, w2s, trace=False)
    return out
